# revision 4
# baseline (speedup 1.0000x reference)
"""Trainium2 Bass kernel for nn_AttentionWithCommunity.

Reference computation (see problem):
    in_active[c] = c in community_index
    comm = node2community[nodes]; use = in_active[comm]
    member_embedding[n] = sum_m score[comm[n],m] * E[neigh_com[comm[n],m]]
      (depends ONLY on comm[n] -> per-community aggregate)
    pred1 = MLP1([node_emb, E[nodes], member_embedding]); pred2 = MLP2(node_emb)
    out = where(use, pred1, pred2)

Strategy (8 cores, single SPMD launch, no collectives):
  Host does integer bookkeeping / layout packing only: active set, valid
  (community, member) pairs, greedy assignment of referenced active
  communities to cores, co-locating every active node with its
  community's core, int16 gather-index packing, one-hot(local comm)
  matrix, transposed node_emb shard.  All embedding-data movement and
  float math runs on device:
    Stage A: one bulk dma_gather of E rows for this core's valid pairs,
             matmul chain with the host-packed score matrix ->
             aggT[d, local_comm], premultiplied by W1's member-slice
             into B3[local_comm, hidden].
    Stage B: bulk dma_gather of node_emb rows (int16 direct) and
             E[nodes] rows (4 range-buckets for int16), PE-transpose to
             feature-major, one-hot matmul against B3 for the member
             term, fused MLP1 -> pred1.
    Stage C: contiguous transposed 1/8 shard of node_emb through
             MLP2 -> pred2 (no transposes, no gathers).
  Host merges where(use, pred1, pred2).
"""

import os
import sys

import numpy as np

for _p in ("/opt/trn_rl_repo", "/root/.axon_site/_ro/trn_rl_repo"):
    if os.path.isdir(_p) and _p not in sys.path:
        sys.path.append(_p)

import concourse.bacc as bacc
import concourse.bass as bass
import concourse.mybir as mybir
from concourse.bass_utils import run_bass_kernel_spmd
from concourse.masks import make_identity
from concourse.tile import TileContext

N, V, C, M, D = 20000, 100000, 5000, 32, 128
NCORES = 8
P = 128
SUP = 4                 # node tiles per matmul supertile (free dim 512)
NW = SUP * P
NBUCK = 4               # E row-range buckets for int16 dma_gather
BUCK = V // NBUCK       # 25000 rows per bucket (< 2**15)

LAST_RESULTS = None     # set by kernel(); test harness reads exec_time_ns


def _roundup(x, m):
    return ((x + m - 1) // m) * m


def _pack_idx16(dst, values, base_slot):
    """Place values into an int16 idx plane [16, S] at linear slots
    base_slot..base_slot+len-1 (slot i -> [i % 16, i // 16])."""
    s = np.arange(base_slot, base_slot + len(values))
    dst[s % 16, s // 16] = values.astype(np.int16)


def _build(NKA, NTB, NKC, NTC, NCOMMP, buck_tiles, b2, b4):
    """Build the per-core SPMD Bass program. All sizes compile-time."""
    f32 = mybir.dt.float32
    i16 = mybir.dt.int16
    NBP = NTB * P
    NCP = NTC * P
    NPAIR = NKA * P

    nc = bacc.Bacc("TRN2", target_bir_lowering=False)
    E_h = nc.dram_tensor("E", [V, D], f32, kind="ExternalInput")
    ne_h = nc.dram_tensor("nodemb", [N, D], f32, kind="ExternalInput")
    csT_h = nc.dram_tensor("cshardT", [P, NCP], f32, kind="ExternalInput")
    ap_h = nc.dram_tensor("a_pack", [P, NKA * NCOMMP], f32, kind="ExternalInput")
    pi_h = nc.dram_tensor("pair_i16", [P, NPAIR // 16], i16, kind="ExternalInput")
    i1_h = nc.dram_tensor("idx1_16", [P, NBP // 16], i16, kind="ExternalInput")
    i2_h = nc.dram_tensor("idx2_16", [P, NBP // 16], i16, kind="ExternalInput")
    oh_h = nc.dram_tensor("oh_pack", [P, NKC * NBP], f32, kind="ExternalInput")
    w1ab_h = nc.dram_tensor("w1ab", [P, 2 * P], f32, kind="ExternalInput")
    w1c_h = nc.dram_tensor("w1c", [P, P], f32, kind="ExternalInput")
    w2_h = nc.dram_tensor("w2", [P, 1], f32, kind="ExternalInput")
    w3_h = nc.dram_tensor("w3", [P, 64], f32, kind="ExternalInput")
    w4_h = nc.dram_tensor("w4", [64, 1], f32, kind="ExternalInput")
    b1_h = nc.dram_tensor("b1c", [P, 1], f32, kind="ExternalInput")
    b3_h = nc.dram_tensor("b3c", [64, 1], f32, kind="ExternalInput")
    p1_h = nc.dram_tensor("pred1", [1, NBP], f32, kind="ExternalOutput")
    p2_h = nc.dram_tensor("pred2", [1, NCP], f32, kind="ExternalOutput")

    Relu = mybir.ActivationFunctionType.Relu
    Ident = mybir.ActivationFunctionType.Identity

    with TileContext(nc) as tc:
        with (
            tc.tile_pool(name="sbc", bufs=1) as sbc,
            tc.tile_pool(name="sbw", bufs=4) as sbw,
            tc.tile_pool(name="pst", bufs=3, space="PSUM") as pst,
            tc.tile_pool(name="psh", bufs=2, space="PSUM") as psh,
            tc.tile_pool(name="psp", bufs=2, space="PSUM") as psp,
        ):
            # ---- constants ----
            ident = sbc.tile([P, P], f32)
            make_identity(nc, ident[:])
            pair_i16 = sbc.tile([P, NPAIR // 16], i16)
            nc.sync.dma_start(out=pair_i16[:], in_=pi_h[:])
            a_pack = sbc.tile([P, NKA * NCOMMP], f32)
            nc.sync.dma_start(out=a_pack[:], in_=ap_h[:])
            idx1_16 = sbc.tile([P, NBP // 16], i16)
            nc.sync.dma_start(out=idx1_16[:], in_=i1_h[:])
            idx2_16 = sbc.tile([P, NBP // 16], i16)
            nc.sync.dma_start(out=idx2_16[:], in_=i2_h[:])
            oh_pack = sbc.tile([P, NKC * NBP], f32)
            nc.sync.dma_start(out=oh_pack[:], in_=oh_h[:])
            w1ab = sbc.tile([P, 2 * P], f32)
            nc.sync.dma_start(out=w1ab[:], in_=w1ab_h[:])
            w1c = sbc.tile([P, P], f32)
            nc.sync.dma_start(out=w1c[:], in_=w1c_h[:])
            w2 = sbc.tile([P, 1], f32)
            nc.sync.dma_start(out=w2[:], in_=w2_h[:])
            w3 = sbc.tile([P, 64], f32)
            nc.sync.dma_start(out=w3[:], in_=w3_h[:])
            w4 = sbc.tile([64, 1], f32)
            nc.sync.dma_start(out=w4[:], in_=w4_h[:])
            b1c = sbc.tile([P, 1], f32)
            nc.sync.dma_start(out=b1c[:], in_=b1_h[:])
            b3c = sbc.tile([64, 1], f32)
            nc.sync.dma_start(out=b3c[:], in_=b3_h[:])
            pred1 = sbc.tile([1, NBP], f32)
            pred2 = sbc.tile([1, NCP], f32)

            # ---- stage A: aggT[d, lc] = sum_pairs E[neigh,d]*score ----
            # dma_gather crashes above 1024 idxs/op -> chunk into <=8 blocks
            GMAX = 8
            aggT = sbc.tile([P, NCOMMP], f32)
            G = sbc.tile([P, NKA, P], f32)
            for c0 in range(0, NKA, GMAX):
                cl = min(GMAX, NKA - c0)
                nc.gpsimd.dma_gather(
                    G[:, c0:c0 + cl, :], E_h[:],
                    pair_i16[:, c0 * 8:(c0 + cl) * 8], cl * P, cl * P, P)
            with tc.tile_pool(name="psa", bufs=1, space="PSUM") as psa:
                aggT_ps = psa.tile([P, NCOMMP], f32, space="PSUM")
                for kt in range(NKA):
                    nc.tensor.matmul(
                        out=aggT_ps[:], lhsT=G[:, kt, :],
                        rhs=a_pack[:, kt * NCOMMP:(kt + 1) * NCOMMP],
                        start=(kt == 0), stop=(kt == NKA - 1))
                nc.vector.tensor_copy(out=aggT[:], in_=aggT_ps[:])

            # B3[lc, h] = (agg @ W1c)[lc, h], per 128-community block
            B3 = sbc.tile([P, NKC * P], f32)
            for j in range(NKC):
                b3p = pst.tile([P, P], f32, space="PSUM", tag="tp")
                nc.tensor.matmul(out=b3p[:], lhsT=aggT[:, j * P:(j + 1) * P],
                                 rhs=w1c[:], start=True, stop=True)
                nc.vector.tensor_copy(out=B3[:, j * P:(j + 1) * P], in_=b3p[:])

            # ---- stage B gathers: all node_emb rows, E rows by bucket ----
            X1 = sbc.tile([P, NTB, P], f32)
            for c0 in range(0, NTB, GMAX):
                cl = min(GMAX, NTB - c0)
                nc.gpsimd.dma_gather(
                    X1[:, c0:c0 + cl, :], ne_h[:],
                    idx1_16[:, c0 * 8:(c0 + cl) * 8], cl * P, cl * P, P)
            X2 = sbc.tile([P, NTB, P], f32)
            t0 = 0
            for b in range(NBUCK):
                nt = buck_tiles[b]
                for c0 in range(t0, t0 + nt, GMAX):
                    cl = min(GMAX, t0 + nt - c0)
                    nc.gpsimd.dma_gather(
                        X2[:, c0:c0 + cl, :], E_h[b * BUCK:(b + 1) * BUCK, :],
                        idx2_16[:, c0 * 8:(c0 + cl) * 8], cl * P, cl * P, P)
                t0 += nt

            # ---- stage B compute: groups of SUP node-tiles ----
            for g in range(NTB // SUP):
                xT = sbw.tile([P, NW], f32, tag="xT")
                yT = sbw.tile([P, NW], f32, tag="yT")
                for j in range(SUP):
                    t = g * SUP + j
                    tp1 = pst.tile([P, P], f32, space="PSUM", tag="tp")
                    nc.tensor.transpose(out=tp1[:], in_=X1[:, t, :],
                                        identity=ident[:])
                    nc.vector.tensor_copy(out=xT[:, j * P:(j + 1) * P], in_=tp1[:])
                    tp2 = pst.tile([P, P], f32, space="PSUM", tag="tp")
                    nc.tensor.transpose(out=tp2[:], in_=X2[:, t, :],
                                        identity=ident[:])
                    nc.vector.tensor_copy(out=yT[:, j * P:(j + 1) * P], in_=tp2[:])

                HT = psh.tile([P, NW], f32, space="PSUM", tag="ht")
                nc.tensor.matmul(out=HT[:], lhsT=w1ab[:, 0:P], rhs=xT[:],
                                 start=True, stop=False)
                nc.tensor.matmul(out=HT[:], lhsT=w1ab[:, P:2 * P], rhs=yT[:],
                                 start=False, stop=False)
                for jc in range(NKC):
                    nc.tensor.matmul(
                        out=HT[:], lhsT=B3[:, jc * P:(jc + 1) * P],
                        rhs=oh_pack[:, jc * NBP + g * NW:jc * NBP + (g + 1) * NW],
                        start=False, stop=(jc == NKC - 1))
                HTs = sbw.tile([P, NW], f32, tag="hts")
                nc.scalar.activation(out=HTs[:], in_=HT[:], func=Relu,
                                     bias=b1c[:, :1], scale=1.0)
                p1p = psp.tile([1, NW], f32, space="PSUM", tag="pp")
                nc.tensor.matmul(out=p1p[:], lhsT=w2[:], rhs=HTs[:],
                                 start=True, stop=True)
                nc.scalar.activation(
                    out=pred1[0:1, g * NW:(g + 1) * NW], in_=p1p[:],
                    func=Ident, bias=float(b2), scale=1.0)

            # ---- stage C: transposed contiguous shard through MLP2 ----
            for g in range(NTC // SUP):
                zT = sbw.tile([P, NW], f32, tag="zT")
                nc.sync.dma_start(out=zT[:], in_=csT_h[:, g * NW:(g + 1) * NW])
                H2 = psh.tile([64, NW], f32, space="PSUM", tag="ht")
                nc.tensor.matmul(out=H2[:], lhsT=w3[:], rhs=zT[:],
                                 start=True, stop=True)
                H2s = sbw.tile([64, NW], f32, tag="h2s")
                nc.scalar.activation(out=H2s[:], in_=H2[:], func=Relu,
                                     bias=b3c[:, :1], scale=1.0)
                p2p = psp.tile([1, NW], f32, space="PSUM", tag="pp")
                nc.tensor.matmul(out=p2p[:], lhsT=w4[:], rhs=H2s[:],
                                 start=True, stop=True)
                nc.scalar.activation(
                    out=pred2[0:1, g * NW:(g + 1) * NW], in_=p2p[:],
                    func=Ident, bias=float(b4), scale=1.0)

            nc.sync.dma_start(out=p1_h[:], in_=pred1[:])
            nc.sync.dma_start(out=p2_h[:], in_=pred2[:])
    nc.compile()
    return nc


def kernel(node_emb, member_score, community_embeddings, W1, b1, W2, b2,
           W3, b3, W4, b4, node2community, community2node, member_num,
           community_index, nodes):
    global LAST_RESULTS

    node_emb = np.ascontiguousarray(np.asarray(node_emb, np.float32))
    member_score = np.asarray(member_score, np.float32)
    E = np.ascontiguousarray(np.asarray(community_embeddings, np.float32))
    W1 = np.asarray(W1, np.float32)
    b1 = np.asarray(b1, np.float32)
    W2 = np.asarray(W2, np.float32)
    b2 = np.asarray(b2, np.float32)
    W3 = np.asarray(W3, np.float32)
    b3 = np.asarray(b3, np.float32)
    W4 = np.asarray(W4, np.float32)
    b4 = np.asarray(b4, np.float32)
    node2community = np.asarray(node2community).astype(np.int64)
    community2node = np.asarray(community2node).astype(np.int64)
    member_num = np.asarray(member_num).astype(np.int64)
    community_index = np.asarray(community_index).astype(np.int64)
    nodes = np.asarray(nodes).astype(np.int64)

    # ---------- host index bookkeeping ----------
    in_active = np.zeros(C, bool)
    in_active[community_index] = True
    comm = node2community[nodes]
    use = in_active[comm]
    neigh_com = node2community[community2node]      # [C, M], values < C
    len_mask = np.arange(M)[None, :] < member_num[:, None]
    valid = len_mask & in_active[neigh_com]
    score = np.where(valid, member_score, 0.0).astype(np.float32)

    active_ids = np.nonzero(use)[0]
    ref_comms = np.unique(comm[active_ids]) if len(active_ids) else np.empty(0, np.int64)

    node_cnt = np.zeros(C, np.int64)
    if len(active_ids):
        np.add.at(node_cnt, comm[active_ids], 1)
    pair_cnt = valid.sum(1)

    # greedy: assign communities to cores balancing node count, then pairs
    comm_core = np.full(C, -1, np.int32)
    core_comms = [[] for _ in range(NCORES)]
    cn = np.zeros(NCORES, np.int64)
    cp = np.zeros(NCORES, np.int64)
    for c in ref_comms[np.argsort(-node_cnt[ref_comms], kind="stable")]:
        k = int(np.lexsort((cp, cn))[0])
        comm_core[c] = k
        core_comms[k].append(int(c))
        cn[k] += node_cnt[c]
        cp[k] += pair_cnt[c]

    # per-core active nodes, reordered by E-row bucket of nodes[id]
    core_nodes = []
    core_bcnt = []
    for k in range(NCORES):
        ids = active_ids[comm_core[comm[active_ids]] == k]
        b = nodes[ids] // BUCK
        order = np.argsort(b, kind="stable")
        ids = ids[order]
        core_nodes.append(ids)
        core_bcnt.append(np.bincount(b[order], minlength=NBUCK))

    core_pairs = []
    for k in range(NCORES):
        cs = np.asarray(core_comms[k], np.int64)
        if len(cs):
            lcs, ms = np.nonzero(valid[cs])
            core_pairs.append((neigh_com[cs[lcs], ms].astype(np.int64),
                               score[cs[lcs], ms], lcs.astype(np.int64)))
        else:
            core_pairs.append((np.empty(0, np.int64), np.empty(0, np.float32),
                               np.empty(0, np.int64)))

    NCOMMP = max(_roundup(max((len(c) for c in core_comms), default=0), P), P)
    NKC = NCOMMP // P
    NKA = max(_roundup(max(len(p[0]) for p in core_pairs), P) // P, 1)
    # per-bucket tile counts shared across cores (SPMD: same shapes)
    buck_tiles = [max(_roundup(int(max(bc[b] for bc in core_bcnt)), P) // P, 0)
                  for b in range(NBUCK)]
    NTB = max(_roundup(sum(buck_tiles), SUP), SUP)
    # pad the last nonempty bucket so sum(buck_tiles) == NTB
    buck_tiles[-1] += NTB - sum(buck_tiles)
    NBP = NTB * P
    NSH = N // NCORES
    NTC = _roundup(NSH, NW) // P
    NCP = NTC * P

    nc = _build(NKA, NTB, NKC, NTC, NCOMMP, buck_tiles,
                float(b2[0]), float(b4[0]))

    # ---------- per-core input packing ----------
    w1ab = np.ascontiguousarray(np.concatenate([W1[0:P], W1[P:2 * P]], axis=1))
    w1c = np.ascontiguousarray(W1[2 * P:3 * P])
    b1c = np.ascontiguousarray(b1[:, None])
    b3c = np.ascontiguousarray(b3[:, None])
    node_embT = np.ascontiguousarray(node_emb.T)

    bstart = np.concatenate([[0], np.cumsum(buck_tiles)]).astype(np.int64)

    in_maps = []
    for k in range(NCORES):
        neigh, sc, lcs = core_pairs[k]
        npair = len(neigh)
        a_pack = np.zeros((P, NKA, NCOMMP), np.float32)
        pp = np.arange(npair)
        a_pack[pp % P, pp // P, lcs] = sc
        pair_i16 = np.zeros((16, NKA * P // 16), np.int16)
        _pack_idx16(pair_i16, neigh, 0)

        ids = core_nodes[k]
        bcnt = core_bcnt[k]
        idx1_16 = np.zeros((16, NBP // 16), np.int16)
        idx2_16 = np.zeros((16, NBP // 16), np.int16)
        oh = np.zeros((NCOMMP, NBP), np.float32)
        lc_of = np.zeros(C, np.int64)
        cs = np.asarray(core_comms[k], np.int64)
        if len(cs):
            lc_of[cs] = np.arange(len(cs))
        # per-bucket slot layout: bucket b occupies slots
        # [bstart[b]*P, bstart[b]*P + bcnt[b])
        slot_of = np.zeros(len(ids), np.int64)
        pos = 0
        for b in range(NBUCK):
            nb = int(bcnt[b])
            if nb == 0:
                continue
            sl0 = int(bstart[b]) * P
            seg = ids[pos:pos + nb]
            slot_of[pos:pos + nb] = sl0 + np.arange(nb)
            _pack_idx16(idx1_16, seg, sl0)
            _pack_idx16(idx2_16, nodes[seg] - b * BUCK, sl0)
            oh[lc_of[comm[seg]], sl0 + np.arange(nb)] = 1.0
            pos += nb
        # one-hot packed as [P, NKC, NBP] partition-major blocks
        oh_pack = np.ascontiguousarray(
            oh.reshape(NKC, P, NBP).transpose(1, 0, 2).reshape(P, NKC * NBP))

        cshardT = np.zeros((P, NCP), np.float32)
        cshardT[:, :NSH] = node_embT[:, k * NSH:(k + 1) * NSH]

        in_maps.append(dict(
            E=E, nodemb=node_emb, cshardT=cshardT,
            a_pack=np.ascontiguousarray(a_pack.reshape(P, NKA * NCOMMP)),
            pair_i16=np.tile(pair_i16, (8, 1)),
            idx1_16=np.tile(idx1_16, (8, 1)),
            idx2_16=np.tile(idx2_16, (8, 1)),
            oh_pack=oh_pack, w1ab=w1ab, w1c=w1c,
            w2=W2, w3=W3, w4=W4, b1c=b1c, b3c=b3c))
        core_nodes[k] = (ids, slot_of)

    res = run_bass_kernel_spmd(nc, in_maps, core_ids=list(range(NCORES)))
    LAST_RESULTS = res

    out = np.empty(N, np.float32)
    for k in range(NCORES):
        out[k * NSH:(k + 1) * NSH] = res.results[k]["pred2"][0, :NSH]
    for k in range(NCORES):
        ids, slot_of = core_nodes[k]
        if len(ids):
            out[ids] = res.results[k]["pred1"][0, slot_of]
    return out


# revision 5
# speedup vs baseline: 1.3924x; 1.3924x over previous
"""Trainium2 Bass kernel for nn_AttentionWithCommunity.

Reference computation (see problem):
    in_active[c] = c in community_index
    comm = node2community[nodes]; use = in_active[comm]
    member_embedding[n] = sum_m score[comm[n],m] * E[neigh_com[comm[n],m]]
      (depends ONLY on comm[n] -> per-community aggregate)
    pred1 = MLP1([node_emb, E[nodes], member_embedding]); pred2 = MLP2(node_emb)
    out = where(use, pred1, pred2)

Strategy (8 cores, single SPMD launch, no collectives):
  Host does integer bookkeeping / layout packing only: active set, valid
  (community, member) pairs (deduped per distinct E row), greedy
  assignment of referenced active communities to cores, co-locating
  every active node with its community's core, one-hot(local comm)
  matrix, transposed node_emb shard.  All embedding-data movement and
  float math runs on device:
    Stage A: indirect-DMA gather of the distinct E rows behind this
             core's valid pairs, matmul chain with the host-packed
             score matrix -> aggT[d, local_comm], premultiplied by W1's
             member-slice into B3[local_comm, hidden].
    Stage B: indirect-DMA gather of node_emb / E[nodes] rows (128/op),
             PE-transpose to feature-major, one-hot matmul against B3
             for the member term (transpose-free), fused MLP1 -> pred1.
    Stage C: contiguous transposed 1/8 shard of node_emb through
             MLP2 -> pred2 (no transposes, no gathers).
  Host merges where(use, pred1, pred2).
"""

import os
import sys

import numpy as np

for _p in ("/opt/trn_rl_repo", "/root/.axon_site/_ro/trn_rl_repo"):
    if os.path.isdir(_p) and _p not in sys.path:
        sys.path.append(_p)

import concourse.bacc as bacc
import concourse.bass as bass
import concourse.mybir as mybir
from concourse.bass_utils import run_bass_kernel_spmd
from concourse.masks import make_identity
from concourse.tile import TileContext

N, V, C, M, D = 20000, 100000, 5000, 32, 128
NCORES = 8
P = 128
SUP = 4                 # node tiles per matmul supertile (free dim 512)
NW = SUP * P

LAST_RESULTS = None     # set by kernel(); test harness reads exec_time_ns


def _roundup(x, m):
    return ((x + m - 1) // m) * m


def _build(NKA, NTB, NKC, NTC, NCOMMP, b2, b4):
    """Build the per-core SPMD Bass program. All sizes compile-time."""
    f32 = mybir.dt.float32
    i32 = mybir.dt.int32
    NBP = NTB * P
    NCP = NTC * P

    nc = bacc.Bacc("TRN2", target_bir_lowering=False)
    E_h = nc.dram_tensor("E", [V, D], f32, kind="ExternalInput")
    ne_h = nc.dram_tensor("nodemb", [N, D], f32, kind="ExternalInput")
    csT_h = nc.dram_tensor("cshardT", [P, NCP], f32, kind="ExternalInput")
    ap_h = nc.dram_tensor("a_pack", [P, NKA * NCOMMP], f32, kind="ExternalInput")
    pi_h = nc.dram_tensor("pair_idx", [P, NKA], i32, kind="ExternalInput")
    i1_h = nc.dram_tensor("idx1", [P, NTB], i32, kind="ExternalInput")
    i2_h = nc.dram_tensor("idx2", [P, NTB], i32, kind="ExternalInput")
    oh_h = nc.dram_tensor("oh_pack", [P, NKC * NBP], f32, kind="ExternalInput")
    w1ab_h = nc.dram_tensor("w1ab", [P, 2 * P], f32, kind="ExternalInput")
    w1c_h = nc.dram_tensor("w1c", [P, P], f32, kind="ExternalInput")
    w2_h = nc.dram_tensor("w2", [P, 1], f32, kind="ExternalInput")
    w3_h = nc.dram_tensor("w3", [P, 64], f32, kind="ExternalInput")
    w4_h = nc.dram_tensor("w4", [64, 1], f32, kind="ExternalInput")
    b1_h = nc.dram_tensor("b1c", [P, 1], f32, kind="ExternalInput")
    b3_h = nc.dram_tensor("b3c", [64, 1], f32, kind="ExternalInput")
    p1_h = nc.dram_tensor("pred1", [1, NBP], f32, kind="ExternalOutput")
    p2_h = nc.dram_tensor("pred2", [1, NCP], f32, kind="ExternalOutput")

    Relu = mybir.ActivationFunctionType.Relu
    IOA = bass.IndirectOffsetOnAxis

    with TileContext(nc) as tc:
        with (
            tc.tile_pool(name="sbc", bufs=1) as sbc,
            tc.tile_pool(name="sbw", bufs=4) as sbw,
            tc.tile_pool(name="pst", bufs=3, space="PSUM") as pst,
            tc.tile_pool(name="psh", bufs=2, space="PSUM") as psh,
            tc.tile_pool(name="psp", bufs=2, space="PSUM") as psp,
        ):
            # ---- constants ----
            ident = sbc.tile([P, P], f32)
            make_identity(nc, ident[:])
            pair_idx = sbc.tile([P, NKA], i32)
            nc.sync.dma_start(out=pair_idx[:], in_=pi_h[:])
            a_pack = sbc.tile([P, NKA * NCOMMP], f32)
            nc.sync.dma_start(out=a_pack[:], in_=ap_h[:])
            idx1 = sbc.tile([P, NTB], i32)
            nc.sync.dma_start(out=idx1[:], in_=i1_h[:])
            idx2 = sbc.tile([P, NTB], i32)
            nc.sync.dma_start(out=idx2[:], in_=i2_h[:])
            oh_pack = sbc.tile([P, NKC * NBP], f32)
            nc.sync.dma_start(out=oh_pack[:], in_=oh_h[:])
            w1ab = sbc.tile([P, 2 * P], f32)
            nc.sync.dma_start(out=w1ab[:], in_=w1ab_h[:])
            w1c = sbc.tile([P, P], f32)
            nc.sync.dma_start(out=w1c[:], in_=w1c_h[:])
            w2 = sbc.tile([P, 1], f32)
            nc.sync.dma_start(out=w2[:], in_=w2_h[:])
            w3 = sbc.tile([P, 64], f32)
            nc.sync.dma_start(out=w3[:], in_=w3_h[:])
            w4 = sbc.tile([64, 1], f32)
            nc.sync.dma_start(out=w4[:], in_=w4_h[:])
            b1c = sbc.tile([P, 1], f32)
            nc.sync.dma_start(out=b1c[:], in_=b1_h[:])
            b3c = sbc.tile([64, 1], f32)
            nc.sync.dma_start(out=b3c[:], in_=b3_h[:])
            pred1 = sbc.tile([1, NBP], f32)
            pred2 = sbc.tile([1, NCP], f32)

            # ---- stage A: aggT[d, lc] = sum_pairs E[row,d]*coef ----
            aggT = sbc.tile([P, NCOMMP], f32)
            with tc.tile_pool(name="psa", bufs=1, space="PSUM") as psa:
                aggT_ps = psa.tile([P, NCOMMP], f32, space="PSUM")
                for kt in range(NKA):
                    g = sbw.tile([P, P], f32, tag="ga")
                    nc.gpsimd.indirect_dma_start(
                        out=g[:], out_offset=None, in_=E_h[:],
                        in_offset=IOA(ap=pair_idx[:, kt:kt + 1], axis=0))
                    nc.tensor.matmul(
                        out=aggT_ps[:], lhsT=g[:],
                        rhs=a_pack[:, kt * NCOMMP:(kt + 1) * NCOMMP],
                        start=(kt == 0), stop=(kt == NKA - 1))
                nc.vector.tensor_copy(out=aggT[:], in_=aggT_ps[:])

            # B3[lc, h] = (agg @ W1c)[lc, h], per 128-community block
            B3 = sbc.tile([P, NKC * P], f32)
            for j in range(NKC):
                b3p = pst.tile([P, P], f32, space="PSUM", tag="tp")
                nc.tensor.matmul(out=b3p[:], lhsT=aggT[:, j * P:(j + 1) * P],
                                 rhs=w1c[:], start=True, stop=True)
                nc.vector.tensor_copy(out=B3[:, j * P:(j + 1) * P], in_=b3p[:])

            # ---- stage B: active nodes, groups of SUP node-tiles ----
            for g in range(NTB // SUP):
                xT = sbw.tile([P, NW], f32, tag="xT")
                yT = sbw.tile([P, NW], f32, tag="yT")
                for j in range(SUP):
                    t = g * SUP + j
                    x1 = sbw.tile([P, P], f32, tag="g1")
                    nc.gpsimd.indirect_dma_start(
                        out=x1[:], out_offset=None, in_=ne_h[:],
                        in_offset=IOA(ap=idx1[:, t:t + 1], axis=0))
                    tp1 = pst.tile([P, P], f32, space="PSUM", tag="tp")
                    nc.tensor.transpose(out=tp1[:], in_=x1[:], identity=ident[:])
                    nc.vector.tensor_copy(out=xT[:, j * P:(j + 1) * P], in_=tp1[:])
                    x2 = sbw.tile([P, P], f32, tag="g2")
                    nc.gpsimd.indirect_dma_start(
                        out=x2[:], out_offset=None, in_=E_h[:],
                        in_offset=IOA(ap=idx2[:, t:t + 1], axis=0))
                    tp2 = pst.tile([P, P], f32, space="PSUM", tag="tp")
                    nc.tensor.transpose(out=tp2[:], in_=x2[:], identity=ident[:])
                    nc.vector.tensor_copy(out=yT[:, j * P:(j + 1) * P], in_=tp2[:])

                HT = psh.tile([P, NW], f32, space="PSUM", tag="ht")
                nc.tensor.matmul(out=HT[:], lhsT=w1ab[:, 0:P], rhs=xT[:],
                                 start=True, stop=False)
                nc.tensor.matmul(out=HT[:], lhsT=w1ab[:, P:2 * P], rhs=yT[:],
                                 start=False, stop=False)
                for jc in range(NKC):
                    nc.tensor.matmul(
                        out=HT[:], lhsT=B3[:, jc * P:(jc + 1) * P],
                        rhs=oh_pack[:, jc * NBP + g * NW:jc * NBP + (g + 1) * NW],
                        start=False, stop=(jc == NKC - 1))
                HTs = sbw.tile([P, NW], f32, tag="hts")
                nc.scalar.activation(out=HTs[:], in_=HT[:], func=Relu,
                                     bias=b1c[:, :1], scale=1.0)
                p1p = psp.tile([1, NW], f32, space="PSUM", tag="pp")
                nc.tensor.matmul(out=p1p[:], lhsT=w2[:], rhs=HTs[:],
                                 start=True, stop=True)
                nc.vector.tensor_scalar_add(
                    out=pred1[0:1, g * NW:(g + 1) * NW], in0=p1p[:], scalar1=float(b2))

            # ---- stage C: transposed contiguous shard through MLP2 ----
            for g in range(NTC // SUP):
                zT = sbw.tile([P, NW], f32, tag="zT")
                nc.sync.dma_start(out=zT[:], in_=csT_h[:, g * NW:(g + 1) * NW])
                H2 = psh.tile([64, NW], f32, space="PSUM", tag="ht")
                nc.tensor.matmul(out=H2[:], lhsT=w3[:], rhs=zT[:],
                                 start=True, stop=True)
                H2s = sbw.tile([64, NW], f32, tag="h2s")
                nc.scalar.activation(out=H2s[:], in_=H2[:], func=Relu,
                                     bias=b3c[:, :1], scale=1.0)
                p2p = psp.tile([1, NW], f32, space="PSUM", tag="pp")
                nc.tensor.matmul(out=p2p[:], lhsT=w4[:], rhs=H2s[:],
                                 start=True, stop=True)
                nc.vector.tensor_scalar_add(
                    out=pred2[0:1, g * NW:(g + 1) * NW], in0=p2p[:], scalar1=float(b4))

            nc.sync.dma_start(out=p1_h[:], in_=pred1[:])
            nc.sync.dma_start(out=p2_h[:], in_=pred2[:])
    nc.compile()
    return nc


def kernel(node_emb, member_score, community_embeddings, W1, b1, W2, b2,
           W3, b3, W4, b4, node2community, community2node, member_num,
           community_index, nodes):
    global LAST_RESULTS

    node_emb = np.ascontiguousarray(np.asarray(node_emb, np.float32))
    member_score = np.asarray(member_score, np.float32)
    E = np.ascontiguousarray(np.asarray(community_embeddings, np.float32))
    W1 = np.asarray(W1, np.float32)
    b1 = np.asarray(b1, np.float32)
    W2 = np.asarray(W2, np.float32)
    b2 = np.asarray(b2, np.float32)
    W3 = np.asarray(W3, np.float32)
    b3 = np.asarray(b3, np.float32)
    W4 = np.asarray(W4, np.float32)
    b4 = np.asarray(b4, np.float32)
    node2community = np.asarray(node2community).astype(np.int64)
    community2node = np.asarray(community2node).astype(np.int64)
    member_num = np.asarray(member_num).astype(np.int64)
    community_index = np.asarray(community_index).astype(np.int64)
    nodes = np.asarray(nodes).astype(np.int64)

    # ---------- host index bookkeeping ----------
    in_active = np.zeros(C, bool)
    in_active[community_index] = True
    comm = node2community[nodes]
    use = in_active[comm]
    neigh_com = node2community[community2node]      # [C, M], values < C
    len_mask = np.arange(M)[None, :] < member_num[:, None]
    valid = len_mask & in_active[neigh_com]
    score = np.where(valid, member_score, 0.0).astype(np.float32)

    active_ids = np.nonzero(use)[0]
    ref_comms = np.unique(comm[active_ids]) if len(active_ids) else np.empty(0, np.int64)

    node_cnt = np.zeros(C, np.int64)
    if len(active_ids):
        np.add.at(node_cnt, comm[active_ids], 1)
    pair_cnt = valid.sum(1)

    # greedy: assign communities to cores balancing node count, then pairs
    comm_core = np.full(C, -1, np.int32)
    core_comms = [[] for _ in range(NCORES)]
    cn = np.zeros(NCORES, np.int64)
    cp = np.zeros(NCORES, np.int64)
    for c in ref_comms[np.argsort(-node_cnt[ref_comms], kind="stable")]:
        k = int(np.lexsort((cp, cn))[0])
        comm_core[c] = k
        core_comms[k].append(int(c))
        cn[k] += node_cnt[c]
        cp[k] += pair_cnt[c]

    core_nodes = [active_ids[comm_core[comm[active_ids]] == k]
                  for k in range(NCORES)]

    # per-core pair lists, deduped by distinct E row: coefficients of
    # duplicate (row, community) pairs add up
    core_pairs = []
    for k in range(NCORES):
        cs = np.asarray(core_comms[k], np.int64)
        if len(cs):
            lcs, ms = np.nonzero(valid[cs])
            rows = neigh_com[cs[lcs], ms]
            sc = score[cs[lcs], ms].astype(np.float64)
            key = rows * NCORES * C + lcs          # unique (row, lc) key
            ukey, inv = np.unique(key, return_inverse=True)
            usc = np.zeros(len(ukey))
            np.add.at(usc, inv, sc)
            urows = ukey // (NCORES * C)
            ulcs = ukey % (NCORES * C)
            # dedup distinct rows for the gather; column index = row slot
            grows, ginv = np.unique(urows, return_inverse=True)
            core_pairs.append((grows, ginv, usc.astype(np.float32), ulcs))
        else:
            core_pairs.append((np.empty(0, np.int64), np.empty(0, np.int64),
                               np.empty(0, np.float32), np.empty(0, np.int64)))

    NCOMMP = max(_roundup(max((len(c) for c in core_comms), default=0), P), P)
    NKC = NCOMMP // P
    NKA = max(_roundup(max(len(p[0]) for p in core_pairs), P) // P, 1)
    NTB = max(_roundup(max((len(n) for n in core_nodes), default=0), NW) // P, SUP)
    NBP = NTB * P
    NSH = N // NCORES
    NTC = _roundup(NSH, NW) // P
    NCP = NTC * P

    nc = _build(NKA, NTB, NKC, NTC, NCOMMP, float(b2[0]), float(b4[0]))

    # ---------- per-core input packing ----------
    w1ab = np.ascontiguousarray(np.concatenate([W1[0:P], W1[P:2 * P]], axis=1))
    w1c = np.ascontiguousarray(W1[2 * P:3 * P])
    b1c = np.ascontiguousarray(b1[:, None])
    b3c = np.ascontiguousarray(b3[:, None])
    node_embT = np.ascontiguousarray(node_emb.T)

    in_maps = []
    for k in range(NCORES):
        grows, ginv, usc, ulcs = core_pairs[k]
        a_pack = np.zeros((P, NKA, NCOMMP), np.float32)
        pair_idx = np.zeros((P, NKA), np.int32)
        gs = np.arange(len(grows))
        pair_idx[gs % P, gs // P] = grows.astype(np.int32)
        # entry for deduped pair j: gather slot ginv[j], community ulcs[j]
        a_pack[ginv % P, ginv // P, ulcs] = usc

        ids = core_nodes[k]
        nb = len(ids)
        idx1 = np.zeros((P, NTB), np.int32)
        idx2 = np.zeros((P, NTB), np.int32)
        oh = np.zeros((NCOMMP, NBP), np.float32)
        lc_of = np.zeros(C, np.int64)
        cs = np.asarray(core_comms[k], np.int64)
        if len(cs):
            lc_of[cs] = np.arange(len(cs))
        ss = np.arange(nb)
        idx1[ss % P, ss // P] = ids.astype(np.int32)
        idx2[ss % P, ss // P] = nodes[ids].astype(np.int32)
        oh[lc_of[comm[ids]], ss] = 1.0
        oh_pack = np.ascontiguousarray(
            oh.reshape(NKC, P, NBP).transpose(1, 0, 2).reshape(P, NKC * NBP))

        cshardT = np.zeros((P, NCP), np.float32)
        cshardT[:, :NSH] = node_embT[:, k * NSH:(k + 1) * NSH]

        in_maps.append(dict(
            E=E, nodemb=node_emb, cshardT=cshardT,
            a_pack=np.ascontiguousarray(a_pack.reshape(P, NKA * NCOMMP)),
            pair_idx=pair_idx, idx1=idx1, idx2=idx2,
            oh_pack=oh_pack, w1ab=w1ab, w1c=w1c,
            w2=W2, w3=W3, w4=W4, b1c=b1c, b3c=b3c))

    res = run_bass_kernel_spmd(nc, in_maps, core_ids=list(range(NCORES)))
    LAST_RESULTS = res

    out = np.empty(N, np.float32)
    for k in range(NCORES):
        out[k * NSH:(k + 1) * NSH] = res.results[k]["pred2"][0, :NSH]
    for k in range(NCORES):
        ids = core_nodes[k]
        if len(ids):
            out[ids] = res.results[k]["pred1"][0, :len(ids)]
    return out


# revision 6
# speedup vs baseline: 1.6422x; 1.1794x over previous
"""Trainium2 Bass kernel for nn_AttentionWithCommunity.

Reference computation (see problem):
    in_active[c] = c in community_index
    comm = node2community[nodes]; use = in_active[comm]
    member_embedding[n] = sum_m score[comm[n],m] * E[neigh_com[comm[n],m]]
      (depends ONLY on comm[n] -> per-community aggregate)
    pred1 = MLP1([node_emb, E[nodes], member_embedding]); pred2 = MLP2(node_emb)
    out = where(use, pred1, pred2)

Strategy (8 cores, single SPMD launch, no collectives):
  Host does integer bookkeeping / layout packing only: active set, valid
  (community, member) pairs (deduped per distinct E row), greedy
  assignment of referenced active communities to cores, co-locating
  every active node with its community's core, one-hot(local comm)
  matrix, transposed node_emb shard.  All embedding-data movement and
  float math runs on device:
    Stage A: indirect-DMA gather of the distinct E rows behind this
             core's valid pairs, matmul chain with the host-packed
             score matrix -> aggT[d, local_comm], premultiplied by W1's
             member-slice into B3[local_comm, hidden].
    Stage B: indirect-DMA gather of node_emb / E[nodes] rows (128/op),
             PE-transpose to feature-major, one-hot matmul against B3
             for the member term (transpose-free), fused MLP1 -> pred1.
    Stage C: contiguous transposed 1/8 shard of node_emb through
             MLP2 -> pred2 (no transposes, no gathers).
  Emission order puts all gathers first and the gather-independent
  stage C right after, so the tensor engine works through MLP2 while
  the GpSimd descriptor-generation for the gathers streams.
  Host merges where(use, pred1, pred2).
"""

import os
import sys

import numpy as np

for _p in ("/opt/trn_rl_repo", "/root/.axon_site/_ro/trn_rl_repo"):
    if os.path.isdir(_p) and _p not in sys.path:
        sys.path.append(_p)

import concourse.bacc as bacc
import concourse.bass as bass
import concourse.mybir as mybir
from concourse.bass_utils import run_bass_kernel_spmd
from concourse.tile import TileContext

N, V, C, M, D = 20000, 100000, 5000, 32, 128
NCORES = 8
P = 128
SUP = 4                 # node tiles per matmul supertile (free dim <= 512)
NW = SUP * P

LAST_RESULTS = None     # set by kernel(); test harness reads exec_time_ns


def _roundup(x, m):
    return ((x + m - 1) // m) * m


def _groups(ntiles):
    """Split ntiles into chunks of at most SUP tiles: [(t0, w), ...]."""
    out = []
    t = 0
    while t < ntiles:
        w = min(SUP, ntiles - t)
        out.append((t, w))
        t += w
    return out


def _build(NKA, NTB, NKC, NTC, NCOMMP, b2, b4):
    """Build the per-core SPMD Bass program. All sizes compile-time."""
    f32 = mybir.dt.float32
    i32 = mybir.dt.int32
    NBP = NTB * P
    NCP = NTC * P

    nc = bacc.Bacc("TRN2", target_bir_lowering=False)
    E_h = nc.dram_tensor("E", [V, D], f32, kind="ExternalInput")
    ne_h = nc.dram_tensor("nodemb", [N, D], f32, kind="ExternalInput")
    csT_h = nc.dram_tensor("cshardT", [P, NCP], f32, kind="ExternalInput")
    ap_h = nc.dram_tensor("a_pack", [P, NKA * NCOMMP], f32, kind="ExternalInput")
    pi_h = nc.dram_tensor("pair_idx", [P, NKA], i32, kind="ExternalInput")
    i1_h = nc.dram_tensor("idx1", [P, NTB], i32, kind="ExternalInput")
    i2_h = nc.dram_tensor("idx2", [P, NTB], i32, kind="ExternalInput")
    oh_h = nc.dram_tensor("oh_pack", [P, NKC * NBP], f32, kind="ExternalInput")
    id_h = nc.dram_tensor("identity", [P, P], f32, kind="ExternalInput")
    w1ab_h = nc.dram_tensor("w1ab", [P, 2 * P], f32, kind="ExternalInput")
    w1c_h = nc.dram_tensor("w1c", [P, P], f32, kind="ExternalInput")
    w2_h = nc.dram_tensor("w2", [P, 1], f32, kind="ExternalInput")
    w3_h = nc.dram_tensor("w3", [P, 64], f32, kind="ExternalInput")
    w4_h = nc.dram_tensor("w4", [64, 1], f32, kind="ExternalInput")
    b1_h = nc.dram_tensor("b1c", [P, 1], f32, kind="ExternalInput")
    b3_h = nc.dram_tensor("b3c", [64, 1], f32, kind="ExternalInput")
    p1_h = nc.dram_tensor("pred1", [1, NBP], f32, kind="ExternalOutput")
    p2_h = nc.dram_tensor("pred2", [1, NCP], f32, kind="ExternalOutput")

    Relu = mybir.ActivationFunctionType.Relu
    IOA = bass.IndirectOffsetOnAxis

    with TileContext(nc) as tc:
        with (
            tc.tile_pool(name="sbc", bufs=1) as sbc,
            tc.tile_pool(name="sbw", bufs=4) as sbw,
            tc.tile_pool(name="pst", bufs=3, space="PSUM") as pst,
            tc.tile_pool(name="psh", bufs=2, space="PSUM") as psh,
            tc.tile_pool(name="psp", bufs=2, space="PSUM") as psp,
        ):
            # ---- constants (small, HWDGE) ----
            pair_idx = sbc.tile([P, NKA], i32)
            nc.sync.dma_start(out=pair_idx[:], in_=pi_h[:])
            idx1 = sbc.tile([P, NTB], i32)
            nc.sync.dma_start(out=idx1[:], in_=i1_h[:])
            idx2 = sbc.tile([P, NTB], i32)
            nc.sync.dma_start(out=idx2[:], in_=i2_h[:])
            ident = sbc.tile([P, P], f32)
            nc.sync.dma_start(out=ident[:], in_=id_h[:])
            w1ab = sbc.tile([P, 2 * P], f32)
            nc.sync.dma_start(out=w1ab[:], in_=w1ab_h[:])
            w1c = sbc.tile([P, P], f32)
            nc.sync.dma_start(out=w1c[:], in_=w1c_h[:])
            w2 = sbc.tile([P, 1], f32)
            nc.sync.dma_start(out=w2[:], in_=w2_h[:])
            w3 = sbc.tile([P, 64], f32)
            nc.sync.dma_start(out=w3[:], in_=w3_h[:])
            w4 = sbc.tile([64, 1], f32)
            nc.sync.dma_start(out=w4[:], in_=w4_h[:])
            b1c = sbc.tile([P, 1], f32)
            nc.sync.dma_start(out=b1c[:], in_=b1_h[:])
            b3c = sbc.tile([64, 1], f32)
            nc.sync.dma_start(out=b3c[:], in_=b3_h[:])
            a_pack = sbc.tile([P, NKA * NCOMMP], f32)
            nc.sync.dma_start(out=a_pack[:], in_=ap_h[:])
            oh_pack = sbc.tile([P, NKC * NBP], f32)
            nc.sync.dma_start(out=oh_pack[:], in_=oh_h[:])
            pred1 = sbc.tile([1, NBP], f32)
            pred2 = sbc.tile([1, NCP], f32)

            # ---- all gathers first: GpSimd streams descriptors while
            # ---- the PE works through gather-independent stage C
            G = sbc.tile([P, NKA, P], f32)
            for kt in range(NKA):
                nc.gpsimd.indirect_dma_start(
                    out=G[:, kt, :], out_offset=None, in_=E_h[:],
                    in_offset=IOA(ap=pair_idx[:, kt:kt + 1], axis=0))
            X1 = sbc.tile([P, NTB, P], f32)
            X2 = sbc.tile([P, NTB, P], f32)
            for t in range(NTB):
                nc.gpsimd.indirect_dma_start(
                    out=X1[:, t, :], out_offset=None, in_=ne_h[:],
                    in_offset=IOA(ap=idx1[:, t:t + 1], axis=0))
                nc.gpsimd.indirect_dma_start(
                    out=X2[:, t, :], out_offset=None, in_=E_h[:],
                    in_offset=IOA(ap=idx2[:, t:t + 1], axis=0))

            # ---- stage C: transposed contiguous shard through MLP2 ----
            for (t0, w) in _groups(NTC):
                fw = w * P
                zT = sbw.tile([P, NW], f32, tag="zT")
                nc.sync.dma_start(out=zT[:, :fw],
                                  in_=csT_h[:, t0 * P:t0 * P + fw])
                H2 = psh.tile([64, NW], f32, space="PSUM", tag="ht")
                nc.tensor.matmul(out=H2[:, :fw], lhsT=w3[:], rhs=zT[:, :fw],
                                 start=True, stop=True)
                H2s = sbw.tile([64, NW], f32, tag="h2s")
                nc.scalar.activation(out=H2s[:, :fw], in_=H2[:, :fw], func=Relu,
                                     bias=b3c[:, :1], scale=1.0)
                p2p = psp.tile([1, NW], f32, space="PSUM", tag="pp")
                nc.tensor.matmul(out=p2p[:, :fw], lhsT=w4[:], rhs=H2s[:, :fw],
                                 start=True, stop=True)
                nc.vector.tensor_scalar_add(
                    out=pred2[0:1, t0 * P:t0 * P + fw], in0=p2p[:, :fw],
                    scalar1=float(b4))
                nc.sync.dma_start(out=p2_h[0:1, t0 * P:t0 * P + fw],
                                  in_=pred2[0:1, t0 * P:t0 * P + fw])

            # ---- stage A: aggT[d, lc] = sum_pairs E[row,d]*coef ----
            aggT = sbc.tile([P, NCOMMP], f32)
            with tc.tile_pool(name="psa", bufs=1, space="PSUM") as psa:
                aggT_ps = psa.tile([P, NCOMMP], f32, space="PSUM")
                for kt in range(NKA):
                    nc.tensor.matmul(
                        out=aggT_ps[:], lhsT=G[:, kt, :],
                        rhs=a_pack[:, kt * NCOMMP:(kt + 1) * NCOMMP],
                        start=(kt == 0), stop=(kt == NKA - 1))
                nc.vector.tensor_copy(out=aggT[:], in_=aggT_ps[:])

            # B3[lc, h] = (agg @ W1c)[lc, h], per 128-community block
            B3 = sbc.tile([P, NKC * P], f32)
            for j in range(NKC):
                b3p = pst.tile([P, P], f32, space="PSUM", tag="tp")
                nc.tensor.matmul(out=b3p[:], lhsT=aggT[:, j * P:(j + 1) * P],
                                 rhs=w1c[:], start=True, stop=True)
                nc.vector.tensor_copy(out=B3[:, j * P:(j + 1) * P], in_=b3p[:])

            # ---- stage B: active nodes, groups of <=SUP node-tiles ----
            for (t0, w) in _groups(NTB):
                fw = w * P
                xT = sbw.tile([P, NW], f32, tag="xT")
                yT = sbw.tile([P, NW], f32, tag="yT")
                for j in range(w):
                    t = t0 + j
                    tp1 = pst.tile([P, P], f32, space="PSUM", tag="tp")
                    nc.tensor.transpose(out=tp1[:], in_=X1[:, t, :],
                                        identity=ident[:])
                    nc.vector.tensor_copy(out=xT[:, j * P:(j + 1) * P], in_=tp1[:])
                    tp2 = pst.tile([P, P], f32, space="PSUM", tag="tp")
                    nc.tensor.transpose(out=tp2[:], in_=X2[:, t, :],
                                        identity=ident[:])
                    nc.vector.tensor_copy(out=yT[:, j * P:(j + 1) * P], in_=tp2[:])

                HT = psh.tile([P, NW], f32, space="PSUM", tag="ht")
                nc.tensor.matmul(out=HT[:, :fw], lhsT=w1ab[:, 0:P],
                                 rhs=xT[:, :fw], start=True, stop=False)
                nc.tensor.matmul(out=HT[:, :fw], lhsT=w1ab[:, P:2 * P],
                                 rhs=yT[:, :fw], start=False, stop=False)
                for jc in range(NKC):
                    nc.tensor.matmul(
                        out=HT[:, :fw], lhsT=B3[:, jc * P:(jc + 1) * P],
                        rhs=oh_pack[:, jc * NBP + t0 * P:jc * NBP + t0 * P + fw],
                        start=False, stop=(jc == NKC - 1))
                HTs = sbw.tile([P, NW], f32, tag="hts")
                nc.scalar.activation(out=HTs[:, :fw], in_=HT[:, :fw], func=Relu,
                                     bias=b1c[:, :1], scale=1.0)
                p1p = psp.tile([1, NW], f32, space="PSUM", tag="pp")
                nc.tensor.matmul(out=p1p[:, :fw], lhsT=w2[:], rhs=HTs[:, :fw],
                                 start=True, stop=True)
                nc.vector.tensor_scalar_add(
                    out=pred1[0:1, t0 * P:t0 * P + fw], in0=p1p[:, :fw],
                    scalar1=float(b2))
                nc.sync.dma_start(out=p1_h[0:1, t0 * P:t0 * P + fw],
                                  in_=pred1[0:1, t0 * P:t0 * P + fw])
    nc.compile()
    return nc


def kernel(node_emb, member_score, community_embeddings, W1, b1, W2, b2,
           W3, b3, W4, b4, node2community, community2node, member_num,
           community_index, nodes):
    global LAST_RESULTS

    node_emb = np.ascontiguousarray(np.asarray(node_emb, np.float32))
    member_score = np.asarray(member_score, np.float32)
    E = np.ascontiguousarray(np.asarray(community_embeddings, np.float32))
    W1 = np.asarray(W1, np.float32)
    b1 = np.asarray(b1, np.float32)
    W2 = np.asarray(W2, np.float32)
    b2 = np.asarray(b2, np.float32)
    W3 = np.asarray(W3, np.float32)
    b3 = np.asarray(b3, np.float32)
    W4 = np.asarray(W4, np.float32)
    b4 = np.asarray(b4, np.float32)
    node2community = np.asarray(node2community).astype(np.int64)
    community2node = np.asarray(community2node).astype(np.int64)
    member_num = np.asarray(member_num).astype(np.int64)
    community_index = np.asarray(community_index).astype(np.int64)
    nodes = np.asarray(nodes).astype(np.int64)

    # ---------- host index bookkeeping ----------
    in_active = np.zeros(C, bool)
    in_active[community_index] = True
    comm = node2community[nodes]
    use = in_active[comm]
    neigh_com = node2community[community2node]      # [C, M], values < C
    len_mask = np.arange(M)[None, :] < member_num[:, None]
    valid = len_mask & in_active[neigh_com]
    score = np.where(valid, member_score, 0.0).astype(np.float32)

    active_ids = np.nonzero(use)[0]
    ref_comms = np.unique(comm[active_ids]) if len(active_ids) else np.empty(0, np.int64)

    node_cnt = np.zeros(C, np.int64)
    if len(active_ids):
        np.add.at(node_cnt, comm[active_ids], 1)
    pair_cnt = valid.sum(1)

    # greedy: assign communities to cores balancing node count, then pairs
    comm_core = np.full(C, -1, np.int32)
    core_comms = [[] for _ in range(NCORES)]
    cn = np.zeros(NCORES, np.int64)
    cp = np.zeros(NCORES, np.int64)
    for c in ref_comms[np.argsort(-node_cnt[ref_comms], kind="stable")]:
        k = int(np.lexsort((cp, cn))[0])
        comm_core[c] = k
        core_comms[k].append(int(c))
        cn[k] += node_cnt[c]
        cp[k] += pair_cnt[c]

    core_nodes = [active_ids[comm_core[comm[active_ids]] == k]
                  for k in range(NCORES)]

    # per-core pair lists, deduped by distinct E row: coefficients of
    # duplicate (row, community) pairs add up
    core_pairs = []
    for k in range(NCORES):
        cs = np.asarray(core_comms[k], np.int64)
        if len(cs):
            lcs, ms = np.nonzero(valid[cs])
            rows = neigh_com[cs[lcs], ms]
            sc = score[cs[lcs], ms].astype(np.float64)
            key = rows * C + lcs                    # unique (row, lc) key
            ukey, inv = np.unique(key, return_inverse=True)
            usc = np.zeros(len(ukey))
            np.add.at(usc, inv, sc)
            urows = ukey // C
            ulcs = ukey % C
            grows, ginv = np.unique(urows, return_inverse=True)
            core_pairs.append((grows, ginv, usc.astype(np.float32), ulcs))
        else:
            core_pairs.append((np.empty(0, np.int64), np.empty(0, np.int64),
                               np.empty(0, np.float32), np.empty(0, np.int64)))

    NCOMMP = max(_roundup(max((len(c) for c in core_comms), default=0), P), P)
    NKC = NCOMMP // P
    NKA = max(_roundup(max(len(p[0]) for p in core_pairs), P) // P, 1)
    NTB = max(_roundup(max((len(n) for n in core_nodes), default=0), P) // P, 1)
    NBP = NTB * P
    NSH = N // NCORES
    NTC = _roundup(NSH, P) // P
    NCP = NTC * P

    nc = _build(NKA, NTB, NKC, NTC, NCOMMP, float(b2[0]), float(b4[0]))

    # ---------- per-core input packing ----------
    w1ab = np.ascontiguousarray(np.concatenate([W1[0:P], W1[P:2 * P]], axis=1))
    w1c = np.ascontiguousarray(W1[2 * P:3 * P])
    b1c = np.ascontiguousarray(b1[:, None])
    b3c = np.ascontiguousarray(b3[:, None])
    identity = np.eye(P, dtype=np.float32)
    node_embT = np.ascontiguousarray(node_emb.T)

    in_maps = []
    for k in range(NCORES):
        grows, ginv, usc, ulcs = core_pairs[k]
        a_pack = np.zeros((P, NKA, NCOMMP), np.float32)
        pair_idx = np.zeros((P, NKA), np.int32)
        gs = np.arange(len(grows))
        pair_idx[gs % P, gs // P] = grows.astype(np.int32)
        a_pack[ginv % P, ginv // P, ulcs] = usc

        ids = core_nodes[k]
        nb = len(ids)
        idx1 = np.zeros((P, NTB), np.int32)
        idx2 = np.zeros((P, NTB), np.int32)
        oh = np.zeros((NCOMMP, NBP), np.float32)
        lc_of = np.zeros(C, np.int64)
        cs = np.asarray(core_comms[k], np.int64)
        if len(cs):
            lc_of[cs] = np.arange(len(cs))
        ss = np.arange(nb)
        idx1[ss % P, ss // P] = ids.astype(np.int32)
        idx2[ss % P, ss // P] = nodes[ids].astype(np.int32)
        oh[lc_of[comm[ids]], ss] = 1.0
        oh_pack = np.ascontiguousarray(
            oh.reshape(NKC, P, NBP).transpose(1, 0, 2).reshape(P, NKC * NBP))

        cshardT = np.zeros((P, NCP), np.float32)
        cshardT[:, :NSH] = node_embT[:, k * NSH:(k + 1) * NSH]

        in_maps.append(dict(
            E=E, nodemb=node_emb, cshardT=cshardT,
            a_pack=np.ascontiguousarray(a_pack.reshape(P, NKA * NCOMMP)),
            pair_idx=pair_idx, idx1=idx1, idx2=idx2,
            oh_pack=oh_pack, identity=identity, w1ab=w1ab, w1c=w1c,
            w2=W2, w3=W3, w4=W4, b1c=b1c, b3c=b3c))

    res = run_bass_kernel_spmd(nc, in_maps, core_ids=list(range(NCORES)))
    LAST_RESULTS = res

    out = np.empty(N, np.float32)
    for k in range(NCORES):
        out[k * NSH:(k + 1) * NSH] = res.results[k]["pred2"][0, :NSH]
    for k in range(NCORES):
        ids = core_nodes[k]
        if len(ids):
            out[ids] = res.results[k]["pred1"][0, :len(ids)]
    return out
